# revision 65
# baseline (speedup 1.0000x reference)
from contextlib import ExitStack

import numpy as np

B, C, H, W = 512, 256, 9, 9
N = H * W
HEADS, KNN = 4, 9
D_K = C // HEADS
E = N * KNN
EPS = 1e-5
NCORES = 8
BPC = B // NCORES
NEG_BIG = -1.0e30

_CACHE = {}


def build_bass(bpc=BPC, ncores=NCORES, stage=6, use_cc=True, csub=3):
    import concourse.bacc as bacc
    import concourse.mybir as mybir
    import concourse.tile as tile
    from concourse.mybir import AluOpType as Op
    from concourse.mybir import ActivationFunctionType as Act

    fp32 = mybir.dt.float32
    f32r = mybir.dt.float32r
    bf16 = mybir.dt.bfloat16
    u16 = mybir.dt.uint16
    ttok = bpc * N

    nc = bacc.Bacc(num_devices=ncores)

    xs = nc.dram_tensor("xs", [bpc, C, N], fp32, kind="ExternalInput")
    wqT = nc.dram_tensor("wqT", [C, C], bf16, kind="ExternalInput")
    wkT = nc.dram_tensor("wkT", [C, C], bf16, kind="ExternalInput")
    wvT = nc.dram_tensor("wvT", [C, C], bf16, kind="ExternalInput")
    bqk = nc.dram_tensor("bqk", [1, C], bf16, kind="ExternalInput")
    ablk = nc.dram_tensor("ablk", [C, 4], bf16, kind="ExternalInput")
    smat = nc.dram_tensor("smat", [N + 1, E], bf16, kind="ExternalInput")
    diagm = nc.dram_tensor("diagm", [N, N], fp32, kind="ExternalInput")
    iota81 = nc.dram_tensor("iota81", [N, 1], u16, kind="ExternalInput")
    iotaf = nc.dram_tensor("iotaf", [N, 1], fp32, kind="ExternalInput")
    gamma = nc.dram_tensor("gamma", [1, C], fp32, kind="ExternalInput")
    beta = nc.dram_tensor("beta", [1, C], fp32, kind="ExternalInput")
    ys = nc.dram_tensor("ys", [bpc, C, N], fp32, kind="ExternalOutput")

    idx_row = nc.dram_tensor("idx_row", [bpc, N, KNN], u16)
    att_scr = nc.dram_tensor("att_scr", [bpc, HEADS, E], bf16)
    cc_in = nc.dram_tensor("cc_in", [128, 4], fp32)
    cc_out = nc.dram_tensor("cc_out", [128, 4], fp32, addr_space="Shared")

    nblk = (bpc + 15) // 16
    BW = 16 * N
    G4 = 4
    ngrp4 = (bpc + G4 - 1) // G4

    with tile.TileContext(nc) as tc:
      with tc.tile_pool(name="consts", bufs=1) as consts:
        diag_sb = consts.tile([N, N], fp32, name="diag_sb")
        nc.sync.dma_start(diag_sb[:, :], diagm[:, :])
        ablk_sb = [consts.tile([128, 4], bf16, name=f"ab{j}") for j in range(2)]
        for j in range(2):
            nc.sync.dma_start(ablk_sb[j][:, :], ablk[j * 128:(j + 1) * 128, :])
        smat_sb = consts.tile([N + 1, E], bf16, name="smat_sb")
        nc.sync.dma_start(smat_sb[:, :], smat[:, :])
        iota_sb = consts.tile([N, 1], u16, name="iota_sb")
        nc.sync.dma_start(iota_sb[:, :], iota81[:, :])
        iotaf_sb = consts.tile([N, 1], fp32, name="iotaf_sb")
        nc.sync.dma_start(iotaf_sb[:, :], iotaf[:, :])
        gam_sb = consts.tile([128, 2], fp32, name="gam_sb")
        bet_sb = consts.tile([128, 2], fp32, name="bet_sb")
        for j in range(2):
            cs = slice(j * 128, (j + 1) * 128)
            nc.sync.dma_start(gam_sb[:, j:j + 1], gamma[:, cs].rearrange("a c -> c a"))
            nc.sync.dma_start(bet_sb[:, j:j + 1], beta[:, cs].rearrange("a c -> c a"))
        nrow = consts.tile([1, N], fp32, name="nrow")
        nc.vector.memset(nrow[:, :], -0.5)
        ones_col = consts.tile([128, 1], fp32, name="ones_col")
        nc.vector.memset(ones_col[:, :], 1.0)

        s_tap = ExitStack()
        tap = s_tap.enter_context(tc.tile_pool(name="tap", bufs=1, side="right"))
        T_all = [[tap.tile([128, min(BW, ttok - blk * BW)], fp32,
                           name=f"T{j}_{blk}") for blk in range(nblk)]
                 for j in range(2)]
        Tb_all = [[tap.tile([128, min(BW, ttok - blk * BW)], bf16,
                            name=f"Tb{j}_{blk}") for blk in range(nblk)]
                  for j in range(2)]

        s_qkv = ExitStack()
        qkv = s_qkv.enter_context(tc.tile_pool(name="qkv", bufs=1))
        qT_sb = qkv.tile([N + 1, bpc * C], bf16, name="qT")
        kT_sb = qkv.tile([N, bpc * C], bf16, name="kT")
        vT_sb = qkv.tile([N, bpc * C], bf16, name="vT")

        with tc.tile_pool(name="napool", bufs=1) as napool, \
             tc.tile_pool(name="workA", bufs=3) as workA, \
             tc.tile_pool(name="wpool", bufs=1) as wpool, \
             tc.tile_pool(name="psA", bufs=2, space="PSUM") as psA, \
             tc.tile_pool(name="psB", bufs=4, space="PSUM") as psB:
            if stage >= 2:
                nc.sync.dma_start(
                    qT_sb[N:N + 1, :].rearrange("p (b c) -> p b c", c=C),
                    bqk[:, :].rearrange("a c -> a () c").to_broadcast([1, bpc, C]))
            w_sb = {}
            for nm, dram in ((("q", wqT), ("k", wkT), ("v", wvT))
                             if stage >= 2 else ()):
                for kc in range(2):
                    t_ = wpool.tile([128, C], bf16, name=f"w{nm}{kc}")
                    nc.sync.dma_start(t_[:, :], dram[kc * 128:(kc + 1) * 128, :])
                    w_sb[(nm, kc)] = t_
            nsq_sb = napool.tile([1, ttok], fp32, name="nsq")
            SW = 432
            for blk in range(nblk):
                nb16 = min(16, bpc - blk * 16)
                for j in range(2):
                    src = (xs[:, j * 128:(j + 1) * 128, :]
                           .rearrange("b c n -> c b n"))
                    bs = slice(16 * blk, 16 * blk + nb16)
                    dst = T_all[j][blk][:, :].rearrange("c (b n) -> c b n", n=N)
                    nc.sync.dma_start(dst[:, :, :], src[:, bs, :])
                    nc.scalar.activation(Tb_all[j][blk][:, :],
                                         T_all[j][blk][:, :], Act.Identity)
                bw = T_all[0][blk].shape[1]
                for t0 in range(0, bw, SW):
                    w_ = min(SW, bw - t0)
                    lo = slice(t0, t0 + w_)
                    gc = slice(blk * BW + t0, blk * BW + t0 + w_)
                    sq_ps = psA.tile([1, SW], fp32, name="sq_ps", tag="sqps",
                                     bufs=1)
                    tsq = workA.tile([128, SW], fp32, name="tsq", tag="tsq")
                    nc.scalar.activation(tsq[:, :w_], T_all[0][blk][:, lo],
                                         Act.Square)
                    tsq2 = workA.tile([128, SW], fp32, name="tsq2", tag="tsq2")
                    nc.scalar.activation(tsq2[:, :w_], T_all[1][blk][:, lo],
                                         Act.Square)
                    nc.vector.tensor_tensor(out=tsq[:, :w_], in0=tsq[:, :w_],
                                            in1=tsq2[:, :w_], op=Op.add)
                    nc.tensor.matmul(sq_ps[:, :w_], ones_col[:, :],
                                     tsq[:, :w_], start=True, stop=True)
                    nc.scalar.activation(nsq_sb[:, gc], sq_ps[:, :w_],
                                         Act.Identity)
                if stage < 1:
                    continue
                idx9g = workA.tile([N, 16 * KNN], u16, name="idx9g", tag="idx9g")
                nc.vector.tensor_copy(
                    idx9g[:, :].rearrange("p (q k) -> p q k", k=KNN)[:, :, 0:1],
                    iota_sb[:, :].rearrange("p a -> p () a")
                        .to_broadcast([N, nb16, 1]))
                for q16 in range(nb16):
                    b = blk * 16 + q16
                    tb = slice(b * N, (b + 1) * N)
                    nd_ps = psA.tile([N, N], fp32, name="nd_ps", tag="ndps",
                                     bufs=3)
                    nc.tensor.matmul(nd_ps[:, :], nrow[:, :], nsq_sb[:, tb],
                                     start=True, stop=False)
                    for j in range(2):
                        ttj = T_all[j][blk]
                        off = (b % 16) * N
                        nc.tensor.matmul(nd_ps[:, :], ttj[:, off:off + N],
                                         ttj[:, off:off + N], start=False,
                                         stop=(j == 1))
                    nd_sb = workA.tile([N, N], fp32, name="nd_sb", tag="ndsb")
                    nc.vector.tensor_tensor(out=nd_sb[:, :], in0=nd_ps[:, :],
                                            in1=diag_sb[:, :], op=Op.add)
                    vals8 = workA.tile([N, 8], fp32, name="vals8", tag="vals8")
                    nc.vector.max(out=vals8[:, :], in_=nd_sb[:, :])
                    nc.vector.max_index(
                        out=idx9g[:, q16 * KNN + 1:(q16 + 1) * KNN],
                        in_max=vals8[:, :], in_values=nd_sb[:, :])
                    nc.scalar.dma_start(
                        idx_row[b, :, :],
                        idx9g[:, q16 * KNN:(q16 + 1) * KNN])
            for b in range(0, bpc if stage >= 2 else 0, 2):
                for nm, dst in (("q", qT_sb), ("k", kT_sb), ("v", vT_sb)):
                    pp = psB.tile([N, 2 * C], fp32, name="pp", tag="projps")
                    for u in range(2):
                        blk, off = (b + u) // 16, ((b + u) % 16) * N
                        for kc in range(2):
                            nc.tensor.matmul(
                                pp[:, u * C:(u + 1) * C],
                                Tb_all[kc][blk][:, off:off + N],
                                w_sb[(nm, kc)][:, :],
                                start=(kc == 0), stop=(kc == 1))
                    use_dve = nm == "q" or (nm == "v" and (b // 2) % 2 == 0)
                    if use_dve:
                        nc.vector.tensor_copy(dst[:N, b * C:(b + 2) * C],
                                              pp[:, :])
                    else:
                        nc.scalar.activation(dst[:N, b * C:(b + 2) * C],
                                             pp[:, :], Act.Identity)

        s_tap.close()

        s_out = ExitStack()
        outp = s_out.enter_context(tc.tile_pool(name="outp", bufs=1, side="right"))
        out_sb = [outp.tile([128, ttok], bf16, name=f"o{j}") for j in range(2)]

        with tc.tile_pool(name="idxp", bufs=3) as idxp, \
             tc.tile_pool(name="ohp", bufs=3) as ohp, \
             tc.tile_pool(name="combp", bufs=3) as combp, \
             tc.tile_pool(name="attp", bufs=3) as attp, \
             tc.tile_pool(name="abcp", bufs=2) as abcp, \
             tc.tile_pool(name="wpp", bufs=3) as wpp, \
             tc.tile_pool(name="psC", bufs=2, space="PSUM") as psC:
            oh_of = {}
            HALVES = ((0, 512), (512, E))

            def pass1_head(g):
                b0 = g * G4
                nb = min(G4, bpc - b0)
                oh_idx = idxp.tile([N, G4 * E], u16, name="oh_idx", tag="ohi")
                nc.gpsimd.dma_start(
                    oh_idx[:, :nb * E],
                    idx_row[b0:b0 + nb, :, :].rearrange("b n k -> () (b n k)")
                        .to_broadcast([N, nb * E]))
                onehot = ohp.tile([N, G4 * E], bf16, name="onehot", tag="oh")
                nc.vector.tensor_scalar(
                    out=onehot[:, :nb * E], in0=oh_idx[:, :nb * E],
                    scalar1=iotaf_sb[:, 0:1], scalar2=None, op0=Op.is_equal)
                sps = psC.tile([128, 1024], fp32, name="sps", tag="sc")
                oh_of[g] = (onehot, sps)
                return onehot, sps

            def pass1_batch(g, bi, onehot, sps):
                b = g * G4 + bi
                comb_sb = []
                for j in range(2):
                    csb = combp.tile([128, E], bf16, name="csb", tag=f"csb{j}")
                    cps = psC.tile([128, 1024], fp32, name="cps", tag="big")
                    for lo, hi in HALVES:
                        er = slice(bi * E + lo, bi * E + hi)
                        nc.tensor.matmul(
                            cps[:, lo:hi],
                            kT_sb[:, b * C + j * 128: b * C + j * 128 + 128],
                            onehot[:, er], start=True, stop=False)
                        nc.tensor.matmul(
                            cps[:, lo:hi],
                            qT_sb[:, b * C + j * 128: b * C + j * 128 + 128],
                            smat_sb[:, lo:hi], start=False, stop=True)
                    nc.scalar.activation(csb[:, :], cps[:, :E],
                                         Act.Prelu, alpha=0.2)
                    comb_sb.append(csb)
                if csub < 2:
                    return
                for j in range(2):
                    for lo, hi in HALVES:
                        nc.tensor.matmul(
                            sps[32 * bi:32 * bi + HEADS, lo:hi],
                            ablk_sb[j][:, :], comb_sb[j][:, lo:hi],
                            start=(j == 0), stop=(j == 1),
                            tile_position=(0, 32 * bi))

            def pass1_tail(g, sps):
                if csub < 2:
                    return
                b0 = g * G4
                nb = min(G4, bpc - b0)
                att = attp.tile([128, E], bf16, name="att", tag="att")
                nc.scalar.activation(att[:, :], sps[:, :E], Act.Exp)
                ssum = attp.tile([128, N], fp32, name="ssum", tag="ssum")
                nc.vector.tensor_reduce(
                    out=ssum[:, :],
                    in_=att[:, :].rearrange("c (n k) -> c n k", k=KNN),
                    axis=mybir.AxisListType.X, op=Op.add)
                sinv = attp.tile([128, N], fp32, name="sinv", tag="sinv")
                nc.vector.reciprocal(out=sinv[:, :], in_=ssum[:, :])
                nc.vector.tensor_tensor(
                    out=att[:, :].rearrange("c (n k) -> c n k", k=KNN),
                    in0=att[:, :].rearrange("c (n k) -> c n k", k=KNN),
                    in1=sinv[:, :].rearrange("c n -> c n ()")
                        .to_broadcast([128, N, KNN]),
                    op=Op.mult)
                for bi in range(nb):
                    nc.sync.dma_start(att_scr[b0 + bi, :, :],
                                      att[32 * bi:32 * bi + HEADS, :])

            abc_of = {}

            def pass2_abc(g):
                b0 = g * G4
                nb = min(G4, bpc - b0)
                abc = []
                for j in range(2):
                    ab = abcp.tile([128, G4 * E], bf16, name="ab", tag=f"ab{j}")
                    for h2 in range(2):
                        nc.sync.dma_start(
                            ab[64 * h2:64 * (h2 + 1), :nb * E]
                                .rearrange("c (b f) -> c b f", f=E),
                            att_scr[b0:b0 + nb, 2 * j + h2, :]
                                .rearrange("b f -> () b f")
                                .to_broadcast([64, nb, E]))
                    abc.append(ab)
                abc_of[g] = abc

            def pass2_batch(g, bi):
                b = g * G4 + bi
                onehot, _ = oh_of[g]
                abc = abc_of[g]
                for j in range(2):
                    wp = wpp.tile([128, E], bf16, name="wp", tag=f"wp{j}")
                    vps = psC.tile([128, 1024], fp32, name="vps", tag="big")
                    for lo, hi in HALVES:
                        er = slice(bi * E + lo, bi * E + hi)
                        nc.tensor.matmul(
                            vps[:, lo:hi],
                            vT_sb[:, b * C + j * 128: b * C + j * 128 + 128],
                            onehot[:, er], start=True, stop=True)
                    vgc = wpp.tile([128, E], bf16, name="vgc", tag=f"vgc{j}",
                                   bufs=2)
                    nc.scalar.activation(vgc[:, :], vps[:, :E], Act.Identity)
                    nc.vector.tensor_tensor(
                        out=wp[:, :], in0=vgc[:, :],
                        in1=abc[j][:, bi * E:(bi + 1) * E], op=Op.mult)
                    wv = wp[:, :].rearrange("c (m k) -> c m k", k=KNN)
                    aa = wpp.tile([128, N * 4], bf16, name="aa", tag=f"aa{j}",
                                  bufs=2)
                    av = aa[:, :].rearrange("c (m k) -> c m k", k=4)
                    nc.vector.tensor_tensor(
                        out=av[:, :, :], in0=wv[:, :, 0:4],
                        in1=wv[:, :, 4:8], op=Op.add)
                    nc.vector.tensor_tensor(
                        out=av[:, :, 0:2], in0=av[:, :, 0:2],
                        in1=av[:, :, 2:4], op=Op.add)
                    nc.vector.tensor_tensor(
                        out=av[:, :, 0:1], in0=av[:, :, 0:1],
                        in1=av[:, :, 1:2], op=Op.add)
                    nc.vector.tensor_tensor(
                        out=out_sb[j][:, b * N:(b + 1) * N]
                            .rearrange("c m -> c m ()"),
                        in0=av[:, :, 0:1], in1=wv[:, :, 8:9], op=Op.add)

            def pass2(g):
                if csub < 3:
                    return
                for bi in range(min(G4, bpc - g * G4)):
                    pass2_batch(g, bi)
                oh_of.pop(g, None)
                abc_of.pop(g, None)

            ng = ngrp4 if stage >= 3 else 0
            for g in range(ng):
                onehot, sps = pass1_head(g)
                if g >= 1 and csub >= 3:
                    pass2_abc(g - 1)
                for bi in range(min(G4, bpc - g * G4)):
                    pass1_batch(g, bi, onehot, sps)
                pass1_tail(g, sps)
                if g >= 1:
                    pass2(g - 1)
            if ng:
                pass2_abc(ng - 1)
                pass2(ng - 1)

        s_qkv.close()

        if stage < 4:
            with tc.tile_pool(name="dzero", bufs=1) as dz:
                z = dz.tile([128, ttok], fp32, name="zt")
                nc.vector.memset(z[:, :], 0.0)
                if stage >= 3 and csub >= 3:
                    for j in range(2):
                        nc.vector.tensor_copy(z[:, :], out_sb[j][:, :])
                for j in range(2):
                    nc.sync.dma_start(
                        ys[:, j * 128:(j + 1) * 128, :].rearrange("b c n -> c b n"),
                        z[:, :].rearrange("c (b n) -> c b n", n=N))
        if stage >= 4:
          with tc.tile_pool(name="dpool", bufs=2) as dpool, \
               tc.tile_pool(name="dwork", bufs=3) as dwork, \
               tc.tile_pool(name="dbig", bufs=1) as dbig:
              xr = [[dbig.tile([128, min(BW, ttok - blk * BW)], fp32,
                               name=f"xr{j}_{blk}") for blk in range(nblk)]
                    for j in range(2)]
              stats = dpool.tile([128, 4], fp32, name="stats")
              sq_sc = dpool.tile([128, 2 * nblk], fp32, name="sq_sc")
              for j in range(2):
                  nc.vector.tensor_reduce(
                      out=stats[:, j:j + 1], in_=out_sb[j][:, :],
                      axis=mybir.AxisListType.X, op=Op.add)
                  for blk in range(nblk):
                      gcol = slice(blk * BW,
                                   min((blk + 1) * BW, ttok))
                      sqp = dwork.tile([128, BW], fp32, name="sqp", tag="sqp")
                      w_ = gcol.stop - gcol.start
                      nc.scalar.activation(sqp[:, :w_], out_sb[j][:, gcol],
                                           Act.Square)
                      nc.vector.tensor_reduce(
                          out=sq_sc[:, j * nblk + blk:j * nblk + blk + 1],
                          in_=sqp[:, :w_],
                          axis=mybir.AxisListType.X, op=Op.add)
              nc.vector.tensor_reduce(
                  out=stats[:, 2:3], in_=sq_sc[:, 0:nblk],
                  axis=mybir.AxisListType.X, op=Op.add)
              nc.vector.tensor_reduce(
                  out=stats[:, 3:4], in_=sq_sc[:, nblk:2 * nblk],
                  axis=mybir.AxisListType.X, op=Op.add)
              nc.sync.dma_start(cc_in[:, :], stats[:, :])
              red = dpool.tile([128, 4], fp32, name="red")
              for j in range(2):
                  src = xs[:, j * 128:(j + 1) * 128, :].rearrange("b c n -> c b n")
                  for blk in range(nblk):
                      hi = min(16 * (blk + 1), bpc)
                      bs = slice(16 * blk, hi)
                      dst = xr[j][blk][:, :].rearrange("c (b n) -> c b n", n=N)
                      nc.scalar.dma_start(dst[:, :, :], src[:, bs, :])
              if ncores > 1 and use_cc:
                  nc.gpsimd.collective_compute(
                      "AllReduce", Op.add,
                      replica_groups=[list(range(ncores))],
                      ins=[cc_in[:, :].opt()], outs=[cc_out[:, :].opt()])
                  nc.sync.dma_start(red[:, :], cc_out[:, :])
              else:
                  nc.sync.dma_start(red[:, :], cc_in[:, :])
              cnt = float(ncores * bpc * N)
              mean = dpool.tile([128, 2], fp32, name="mean")
              nc.vector.tensor_scalar_mul(mean[:, :], red[:, 0:2], 1.0 / cnt)
              ex2 = dpool.tile([128, 2], fp32, name="ex2")
              nc.vector.tensor_scalar_mul(ex2[:, :], red[:, 2:4], 1.0 / cnt)
              var = dpool.tile([128, 2], fp32, name="var")
              nc.vector.tensor_tensor(out=var[:, :], in0=mean[:, :],
                                      in1=mean[:, :], op=Op.mult)
              nc.vector.tensor_tensor(out=var[:, :], in0=ex2[:, :],
                                      in1=var[:, :], op=Op.subtract)
              nc.vector.tensor_scalar_add(var[:, :], var[:, :], EPS)
              lnv = dpool.tile([128, 2], fp32, name="lnv")
              nc.scalar.activation(lnv[:, :], var[:, :], Act.Ln, bias=0.0,
                                   scale=1.0)
              rstd = dpool.tile([128, 2], fp32, name="rstd")
              nc.scalar.activation(rstd[:, :], lnv[:, :], Act.Exp,
                                   bias=0.0, scale=-0.5)
              scale_t = dpool.tile([128, 2], fp32, name="scale_t")
              nc.vector.tensor_tensor(out=scale_t[:, :], in0=gam_sb[:, :],
                                      in1=rstd[:, :], op=Op.mult)
              shift_t = dpool.tile([128, 2], fp32, name="shift_t")
              nc.vector.tensor_tensor(out=shift_t[:, :], in0=mean[:, :],
                                      in1=scale_t[:, :], op=Op.mult)
              nc.vector.tensor_tensor(out=shift_t[:, :], in0=bet_sb[:, :],
                                      in1=shift_t[:, :], op=Op.subtract)
              for j in range(2):
                  dst = ys[:, j * 128:(j + 1) * 128, :].rearrange("b c n -> c b n")
                  for blk in range(nblk):
                      hi = min(16 * (blk + 1), bpc)
                      bs = slice(16 * blk, hi)
                      gcol = slice(blk * BW, blk * BW + (hi - 16 * blk) * N)
                      w_ = gcol.stop - gcol.start
                      yp = dwork.tile([128, BW], fp32, name="yp", tag="yp")
                      nc.vector.scalar_tensor_tensor(
                          out=yp[:, :w_], in0=out_sb[j][:, gcol],
                          scalar=scale_t[:, j:j + 1], in1=xr[j][blk][:, :],
                          op0=Op.mult, op1=Op.add)
                      nc.scalar.activation(yp[:, :w_], yp[:, :w_],
                                           Act.Relu, bias=shift_t[:, j:j + 1],
                                           scale=1.0)
                      srcv = yp[:, :w_].rearrange("c (b n) -> c b n", n=N)
                      if j == 0:
                          nc.sync.dma_start(dst[:, bs, :], srcv[:, :, :])
                      else:
                          nc.scalar.dma_start(dst[:, bs, :], srcv[:, :, :])

        s_out.close()

    nc.finalize()
    return nc


def host_prep(wq, bq, wk, bk, wv, bv, a, gamma, beta):
    import ml_dtypes
    bf16 = ml_dtypes.bfloat16
    ablk = np.zeros((C, 4), np.float32)
    for h in range(HEADS):
        ablk[h * D_K:(h + 1) * D_K, h] = a[h]
    smat = np.zeros((N + 1, E), np.float32)
    for n in range(N):
        smat[n, n * KNN:(n + 1) * KNN] = 1.0
    smat[N, :] = 1.0
    diagm = np.where(np.eye(N, dtype=bool), np.float32(NEG_BIG), np.float32(0.0))
    iota = np.arange(N, dtype=np.uint16).reshape(N, 1)
    return {
        "wqT": np.ascontiguousarray(wq.T).astype(bf16),
        "wkT": np.ascontiguousarray(wk.T).astype(bf16),
        "wvT": np.ascontiguousarray(wv.T).astype(bf16),
        "bqk": (bq + bk).reshape(1, C).astype(bf16),
        "ablk": ablk.astype(bf16),
        "smat": smat.astype(bf16),
        "diagm": diagm.astype(np.float32),
        "iota81": iota,
        "iotaf": np.arange(N, dtype=np.float32).reshape(N, 1),
        "gamma": gamma.reshape(1, C).astype(np.float32),
        "beta": beta.reshape(1, C).astype(np.float32),
    }


def kernel(x, wq, bq, wk, bk, wv, bv, a, gamma, beta):
    from concourse import bass_utils

    x = np.ascontiguousarray(np.asarray(x, np.float32))
    shared = host_prep(*(np.asarray(t, np.float32) for t in
                         (wq, bq, wk, bk, wv, bv, a, gamma, beta)))

    if "nc" not in _CACHE:
        _CACHE["nc"] = build_bass()
    nc = _CACHE["nc"]

    xf = x.reshape(B, C, N)
    in_maps = []
    for core in range(NCORES):
        m = dict(shared)
        m["xs"] = np.ascontiguousarray(xf[core * BPC:(core + 1) * BPC])
        in_maps.append(m)

    res = bass_utils.run_bass_kernel_spmd(nc, in_maps, core_ids=list(range(NCORES)))
    y = np.empty((B, C, N), np.float32)
    for core in range(NCORES):
        y[core * BPC:(core + 1) * BPC] = res.results[core]["ys"]
    return y.reshape(B, C, H, W)
